# revision 25
# baseline (speedup 1.0000x reference)
"""MoE feed-forward (top-2 routing + shared expert) on 8 Trainium2 cores.

Strategy (expert parallel):
  - Host computes the router (tiny [T,D]@[D,E] matmul), top-2 expert ids and
    renormalized gates, then dispatches each expert's tokens (transposed,
    capacity-padded) to the core that owns that expert's weights.
  - Core e computes  ye = (silu(xe@w1_e) * (xe@w3_e)) @ w2_e, row-scaled by the
    gate, plus a 1/8 token-slice of the always-active shared expert.
  - Host scatter-adds routed outputs into the shared-expert output.

All matmul operands are bf16 (fp32 PSUM accumulation).  bf16 keeps the PE at
the full 1 column/cycle rate, enables fast-weight-load (the fp32 LDWEIGHTS
floor of ~190ns is what throttled the fp32r version), and halves HBM traffic.
End-to-end absmax relative error vs the fp32 reference is ~4e-3.

Structure per block (routed C rows, then shared S rows):
  phase 1: for each of 16 h-blocks, stream the [128, 2w*8k*128] w1/w3 tile
    from HBM (read exactly once), matmul against the SBUF-resident activations
    in ~374-wide column chunks (PSUM-bank sized), then silu*mul into a
    bf16 gT buffer.  LDWEIGHTS (~97ns with FWL) hides under the ~158ns MMs.
  phase 2: down-projection: gT token-tiles stationary, w2 (SBUF-resident,
    loaded once) moving in 512-wide slices; gate-scale on the vector engine,
    DMA out per 128-token x 512-col tile.

The shared-expert block runs first (its activation load is 2x smaller so the
PE unblocks sooner); the routed block's inputs stream in behind it.  A few
dummy matmuls on a zeroed tile warm the PE's HAM clock gate during the
initial DMA wait.  Outputs are written as bf16 full-128-row tiles (partial-
height HBM writes serialize onto one SDMA engine, ~15x slower).
"""

import numpy as np
import ml_dtypes

import concourse.bass as bass
import concourse.mybir as mybir
import concourse.tile as tile
from concourse import bacc
from concourse.bass_utils import run_bass_kernel_spmd

P = 128
N_CORES = 8
F32 = mybir.dt.float32
BF16 = mybir.dt.bfloat16
AF = mybir.ActivationFunctionType
NPBF16 = ml_dtypes.bfloat16


def _chunk_widths(n):
    """Split n columns into equal-ish PSUM-bank chunks (<=512 wide).

    Any chunk >= ~240 wide keeps the matmul (w/2.4GHz) above the bf16
    LDWEIGHTS floor (~97ns), so the PE runs at the full 1 col/cycle rate."""
    if n <= 512:
        return [n]
    k = -(-n // 384)
    base, rem = divmod(n, k)
    return [base + (1 if i < rem else 0) for i in range(k)]


def _swiglu_block(
    nc, pools, xT_ap, n_rows, w13_ap, w2_ap, out_ap, ge_tile, use_silu, deferred
):
    """Emit one SwiGLU y = (silu(x@w1) * (x@w3)) @ w2 over n_rows tokens.

    DMA emission order IS the (serial, ~260GB/s) sync-ring execution order, so
    the critical chain goes first: x k=0 slice, h-block 0's w1 half, x k=1,2,
    the w3 half, the remaining x k-slices (h-block 0 consumes them at ~1 per
    us), then the weight stream.  w2 streams in quarters behind the tail of
    the weight stream (it would starve the stream if issued earlier, and it
    is only needed in phase 2).
    deferred: DMA-emission thunks run right after the first x/w DMAs."""
    D = out_ap.shape[1]
    KD = xT_ap.shape[1] // n_rows
    KH = w2_ap.shape[1] // D
    ND = D // 512

    px, pw, pw2, pg, pot, psp, pps1, pps3, ppo = pools

    # activations: SBUF-resident for the whole block, loaded per k-slice so
    # the first matmuls only wait on the slices they touch
    xt = px.tile([P, KD, n_rows], BF16, tag="xt", name="xt")
    nc.sync.dma_start(xt[:, 0, :], xT_ap[:, :n_rows])

    gt = pg.tile([P, KH, n_rows], BF16, tag="gt", name="gt")
    w2t = pw2.tile([P, KH, D], BF16, tag="w2t", name="w2t")

    chunks = _chunk_widths(n_rows)
    wsz = 2 * KD * P  # packed w1/w3 cols per h-block

    # ---- phase 1: gt[h, c] = silu(x@w1) * (x@w3), w1/w3 streamed once ----
    for hb in range(KH):
        wt = pw.tile([P, 2, KD, P], BF16, tag="wt", name="wt")
        for w in range(2):  # w1 half first: chunk 0's k-loop needs only it
            nc.sync.dma_start(
                wt[:, w],
                w13_ap[
                    :, hb * wsz + w * KD * P : hb * wsz + (w + 1) * KD * P
                ].rearrange("p (k c) -> p k c", k=KD),
            )
            if hb == 0 and w == 0:  # x k=1,2 land before the w3 half
                for k in (1, 2):
                    nc.sync.dma_start(
                        xt[:, k, :], xT_ap[:, k * n_rows : (k + 1) * n_rows]
                    )
        if hb == 0:
            for k in range(3, KD):
                nc.sync.dma_start(
                    xt[:, k, :], xT_ap[:, k * n_rows : (k + 1) * n_rows]
                )
            for thunk in deferred:
                thunk()
        if hb >= KH - 4:
            # w2 is needed only in phase 2 -- stream it in quarters behind
            # the tail of the weight stream so the stream isn't starved and
            # the first phase-2 matmuls (kh ascending) unblock in order
            q = KH // 4
            qi = hb - (KH - 4)
            nc.sync.dma_start(
                w2t[:, qi * q : (qi + 1) * q, :],
                w2_ap[:, qi * q * D : (qi + 1) * q * D].rearrange(
                    "p (k m) -> p k m", k=q
                ),
            )
        c0 = 0
        for cw in chunks:
            p1 = pps1.tile([P, 512], F32, tag="p1", name="p1")[:, :cw]
            p3 = pps3.tile([P, 512], F32, tag="p3", name="p3")[:, :cw]
            for k in range(KD):
                nc.tensor.matmul(
                    p1,
                    wt[:, 0, k, :],
                    xt[:, k, c0 : c0 + cw],
                    start=(k == 0),
                    stop=(k == KD - 1),
                )
            for k in range(KD):
                nc.tensor.matmul(
                    p3,
                    wt[:, 1, k, :],
                    xt[:, k, c0 : c0 + cw],
                    start=(k == 0),
                    stop=(k == KD - 1),
                )
            gs = gt[:, hb, c0 : c0 + cw]
            if use_silu:
                nc.scalar.activation(gs, p1, AF.Silu)
                nc.vector.tensor_mul(gs, gs, p3)
            else:  # silu(a) = a * sigmoid(a); CoreSim has no Silu table
                s1 = psp.tile([P, 512], F32, tag="s1", name="s1")[:, :cw]
                nc.scalar.activation(s1, p1, AF.Sigmoid)
                nc.vector.tensor_mul(gs, p1, p3)
                nc.vector.tensor_mul(gs, gs, s1)
            c0 += cw

    # ---- phase 2: out = gate * (gt.T @ w2), token-tiles stationary ----
    for ct in range(-(-n_rows // P)):
        rows = min(P, n_rows - ct * P)
        pos = [ppo.tile([P, 512], F32, tag="po", name="po") for _ in range(ND)]
        for kh in range(KH):
            lhs = gt[:, kh, ct * P : ct * P + rows]
            for dn in range(ND):
                nc.tensor.matmul(
                    pos[dn][:rows],
                    lhs,
                    w2t[:, kh, dn * 512 : (dn + 1) * 512],
                    start=(kh == 0),
                    stop=(kh == KH - 1),
                )
        # bf16 output tile covering the full D row: HBM writes are latency-
        # bound per packet, so one 2KB-per-partition transfer per token tile
        # (instead of two 1KB ones) halves the exposed write tail.  Always
        # write all 128 partitions -- a partial-height transfer lands on a
        # single SDMA engine and runs ~15x slower; out_ap is row-padded and
        # the host ignores rows past the real token count (the padding rows
        # carry gate-zeroed garbage).
        ot = pot.tile([P, ND * 512], BF16, tag="ot", name="ot")
        for dn in range(ND):
            osl = ot[:, dn * 512 : (dn + 1) * 512]
            if dn % 2:  # scalar engine drains the odd bank in parallel
                if ge_tile is not None:
                    nc.scalar.activation(
                        osl, pos[dn], AF.Copy, scale=ge_tile[:, ct : ct + 1]
                    )
                else:
                    nc.scalar.activation(osl, pos[dn], AF.Copy)
            elif ge_tile is not None:
                nc.vector.tensor_scalar_mul(
                    osl, pos[dn], ge_tile[:, ct : ct + 1]
                )
            else:
                nc.vector.tensor_copy(osl, pos[dn])
        nc.sync.dma_start(out_ap[ct * P : (ct + 1) * P, :], ot[:])


def build_moe_program(D, H, C, S, use_silu=True):
    """SPMD program: routed expert over C capacity rows + shared expert over
    S token-slice rows. Same NEFF on all 8 cores, per-core input data."""
    nc = bacc.Bacc(
        "TRN2", target_bir_lowering=False, debug=False, num_devices=N_CORES
    )
    KD = D // P
    KH = H // P
    CP = -(-C // P)

    def din(name, shape, dt=BF16):
        return nc.dram_tensor(name, shape, dt, kind="ExternalInput").ap()

    def dout(name, shape):
        return nc.dram_tensor(name, shape, BF16, kind="ExternalOutput").ap()

    xeT = din("xeT", [P, KD * C])
    ge = din("ge", [P, CP], F32)
    xsT = din("xsT", [P, KD * S])
    w13 = din("w13", [P, KH * KD * 2 * P])
    w2 = din("w2", [P, KH * D])
    sw13 = din("sw13", [P, KH * KD * 2 * P])
    sw2 = din("sw2", [P, KH * D])
    ye = dout("ye", [CP * P, D])  # row-padded: phase 2 writes full tiles
    se = dout("se", [S, D])

    with tile.TileContext(nc) as tc:
        from contextlib import ExitStack

        with ExitStack() as ctx:
            pools = (
                ctx.enter_context(tc.tile_pool(name="x", bufs=2)),
                ctx.enter_context(tc.tile_pool(name="wstream", bufs=4)),
                ctx.enter_context(tc.tile_pool(name="w2res", bufs=2)),
                ctx.enter_context(tc.tile_pool(name="gT", bufs=1)),
                ctx.enter_context(tc.tile_pool(name="otile", bufs=4)),
                ctx.enter_context(tc.tile_pool(name="stemp", bufs=2)),
                ctx.enter_context(tc.tile_pool(name="ps1", bufs=2, space="PSUM")),
                ctx.enter_context(tc.tile_pool(name="ps3", bufs=2, space="PSUM")),
                ctx.enter_context(tc.tile_pool(name="pso", bufs=4, space="PSUM")),
            )
            pge = ctx.enter_context(tc.tile_pool(name="gate", bufs=1))
            pdum = ctx.enter_context(tc.tile_pool(name="dummy", bufs=1))

            # HAM warmup: the PE's clock gate needs ~3.4us of sustained
            # activity to lift the 1.2GHz cold throttle.  A few matmuls on a
            # zeroed tile (results never read) bridge the initial DMA wait so
            # real matmuls start at (or near) full rate.  gpsimd does the
            # memset -- it exits the kernel-entry preamble first.
            dum = pdum.tile([P, 512], BF16, tag="dum", name="dum")
            nc.gpsimd.memset(dum[:], 0.0)
            pdm = pools[6].tile([P, 512], F32, tag="p1", name="pdm")
            for i in range(7):
                nc.tensor.matmul(pdm[:], dum[:, :P], dum[:])

            # shared-expert block first: its activation load is 2x smaller,
            # so the PE unblocks sooner; the routed block's larger inputs
            # stream in behind it during ~80us of shared-expert matmuls.
            _swiglu_block(
                nc, pools, xsT, S, sw13, sw2, se, None, use_silu, []
            )
            get = pge.tile([P, CP], F32, tag="ge", name="get")
            deferred = [lambda: nc.sync.dma_start(get[:], ge)]
            _swiglu_block(
                nc, pools, xeT, C, w13, w2, ye, get, use_silu, deferred
            )

    nc.compile()
    return nc


_PROGRAM_CACHE = {}
LAST_RESULTS = None  # BassKernelResults of the most recent device run (for test.py)


def _get_program(D, H, C, S):
    key = (D, H, C, S)
    if key not in _PROGRAM_CACHE:
        _PROGRAM_CACHE[key] = build_moe_program(D, H, C, S)
    return _PROGRAM_CACHE[key]


def _pack_xT(xmat):
    """[n, D] row-major bf16 tokens -> [P, KD*n] partition-major, k-major."""
    n, Dm = xmat.shape
    KD = Dm // P
    return np.ascontiguousarray(
        xmat.reshape(n, KD, P).transpose(2, 1, 0).reshape(P, KD * n)
    )


def _pack_w13(w1, w3):
    """Two [D, H] bf16 -> [P, KH*2*KD*P]: h-block-major, then w, k, cols."""
    Dw, Hw = w1.shape
    KD = Dw // P
    KH = Hw // P
    a = np.stack([w1, w3], axis=0)  # [w, D, H]
    a = a.reshape(2, KD, P, KH, P).transpose(2, 3, 0, 1, 4)  # [p, hb, w, k, c]
    return np.ascontiguousarray(a.reshape(P, KH * 2 * KD * P))


def _pack_w2(w):
    """[H, D] bf16 -> [P, KH*D], kh-major."""
    Hw, Dw = w.shape
    KH = Hw // P
    return np.ascontiguousarray(
        w.reshape(KH, P, Dw).transpose(1, 0, 2).reshape(P, KH * Dw)
    )


def _route(xf, w_router):
    """Top-2 routing identical (up to fp rounding) to the jax reference."""
    logits = xf @ w_router.astype(np.float32)  # [T, E]
    # softmax is monotone: top-2 of probs == top-2 of logits, stable ties
    top2 = np.argsort(-logits, axis=1, kind="stable")[:, :2]  # [T, 2]
    lv = np.take_along_axis(logits, top2, axis=1)
    ev = np.exp(lv - lv[:, 0:1])
    gates = ev / ev.sum(axis=1, keepdims=True)  # [T, 2] renormalized
    return top2, gates


def kernel(x, w_router, w1, w3, w2, sw1, sw3, sw2):
    B, SEQ, D = x.shape
    T = B * SEQ
    E, _, H = w1.shape
    assert E == N_CORES
    S = T // N_CORES

    x = np.asarray(x, dtype=np.float32)
    xf = np.ascontiguousarray(x.reshape(T, D))
    top2, gates = _route(xf, np.asarray(w_router, np.float32))

    # per-expert token lists + gate values
    flat_e = top2.ravel()  # slot 2t, 2t+1 -> token t
    flat_g = gates.ravel().astype(np.float32)
    order = np.argsort(flat_e, kind="stable")
    sorted_e = flat_e[order]
    starts = np.searchsorted(sorted_e, np.arange(E + 1))
    tok_by_e = [order[starts[e] : starts[e + 1]] >> 1 for e in range(E)]
    gate_by_e = [flat_g[order[starts[e] : starts[e + 1]]] for e in range(E)]
    counts = np.diff(starts)

    # capacity = the actual max expert load (static shapes; the program is
    # cached per C so repeated calls with the same routing reuse the NEFF)
    C = max(int(counts.max()), 512)
    CP = -(-C // P)

    nc = _get_program(D, H, C, S)

    xb = xf.astype(NPBF16)
    w1 = np.asarray(w1, np.float32).astype(NPBF16)
    w3 = np.asarray(w3, np.float32).astype(NPBF16)
    w2 = np.asarray(w2, np.float32).astype(NPBF16)
    sw13p = _pack_w13(
        np.asarray(sw1, np.float32).astype(NPBF16),
        np.asarray(sw3, np.float32).astype(NPBF16),
    )
    sw2p = _pack_w2(np.asarray(sw2, np.float32).astype(NPBF16))

    in_maps = []
    for e in range(E):
        n_e = int(counts[e])
        xe_pad = np.zeros((C, D), NPBF16)
        xe_pad[:n_e] = xb[tok_by_e[e]]
        ge = np.zeros((CP, P), np.float32)
        ge.reshape(-1)[:n_e] = gate_by_e[e]
        in_maps.append(
            {
                "xeT": _pack_xT(xe_pad),
                "ge": np.ascontiguousarray(ge.T),
                "xsT": _pack_xT(xb[e * S : (e + 1) * S]),
                "w13": _pack_w13(w1[e], w3[e]),
                "w2": _pack_w2(w2[e]),
                "sw13": sw13p,
                "sw2": sw2p,
            }
        )

    global LAST_RESULTS
    LAST_RESULTS = run_bass_kernel_spmd(nc, in_maps, core_ids=list(range(N_CORES)))
    res = LAST_RESULTS.results

    out = np.empty((T, D), np.float32)
    for c in range(N_CORES):
        out[c * S : (c + 1) * S] = res[c]["se"].astype(np.float32)
    for e in range(E):
        n_e = int(counts[e])
        if n_e:
            out[tok_by_e[e]] += res[e]["ye"][:n_e].astype(np.float32)
    return out.reshape(B, SEQ, D)
